# revision 29
# baseline (speedup 1.0000x reference)
"""EMD loss kernel for Trainium2 (8 NeuronCores, pure data parallel).

Computes out[b] = sum_t (cumsum(x-y, axis=1)[b, t])^2 for x, y [131072, 256] f32.

Transposed matmul design (v3). The row-major scan design was a three-way tie
(DVE tensor_tensor_scan 667ns/block = 85us, ACT square+accum 739ns/block =
95us, f32 DMA 83us, exec 101us). This version:
  - uploads x and -y as fp16 in a bins-on-partitions transposed layout
    (16.8 MB/core, one packed HWDGE DMA per 2048-row strip).
  - DVE pre-adds z = x + (-y) in fp16 2x mode (z1 = bins 0..127 on the 128
    partitions, z2 = bins 128..255).
  - PE computes the cumulative-sum differences as triangular matmuls
    (C1 = U^T z1; C2 = U^T z2 + ONES^T z1; 3 matmuls per 512-row chunk),
    replacing the unscalable DVE scan.
  - ACT squares two chunks at a time PSUM->SBUF fp16 ([128, 1024] tiles to
    amortize the 172-cycle PSUM access bubble).
  - PE ones-vector matmuls reduce over bins into [1, 512] PSUM rows; two
    chunks share one S bank at partition offsets {0, 64}, halving the DVE
    PSUM->SBUF copies. A single final DMA scatters the staging tile to DRAM.
"""

import numpy as np

from concourse import bacc, bass, mybir
from concourse.bass_utils import run_bass_kernel_spmd
from concourse.masks import make_upper_triangular
from concourse.tile import TileContext

N_CORES = 8
B = 131072
BINS = 256
ROWS = B // N_CORES  # 16384 rows per core
P = 128
CH = 2048  # main strip width (rows per input DMA)
# Tapered strips: small head so compute starts early, small tails so the
# serial post-last-DMA compute is short. All multiples of SUP.
STRIPS = [2048] * 6 + [1024] * 4
assert sum(STRIPS) == ROWS
NCH = 512  # matmul moving free dim (chunk)
SUP = 1024  # super-chunk: ACT square granularity (2 chunks)
N_SUP = ROWS // SUP  # 16

F32 = mybir.dt.float32
F16 = mybir.dt.float16


def build_nc() -> bass.Bass:
    nc = bacc.Bacc()

    # Strip-major host layout: per (partition, strip) all four quadrants
    # (x/ny x binhalf) are contiguous, so each strip DMA is one long run
    # per partition instead of four short ones.
    xy = nc.declare_dram_parameter("xy", [P, 4 * ROWS], F16, isOutput=False)
    out = nc.declare_dram_parameter("out", [ROWS], F32, isOutput=True)
    xv = xy[:]

    with (
        TileContext(nc) as tc,
        tc.tile_pool(name="io", bufs=5) as io_pool,
        tc.tile_pool(name="zp", bufs=2) as z_pool,
        tc.tile_pool(name="sq", bufs=2) as sq_pool,
        tc.tile_pool(name="c1p", bufs=3, space="PSUM") as c1_pool,
        tc.tile_pool(name="c2p", bufs=3, space="PSUM") as c2_pool,
        tc.tile_pool(name="sp", bufs=2, space="PSUM") as s_pool,
        tc.tile_pool(name="const", bufs=1) as const_pool,
    ):
        U = const_pool.tile([P, P], F16, tag="U")
        ONES = const_pool.tile([P, P], F16, tag="ONES")
        stage = const_pool.tile([P, N_SUP, NCH], F32, tag="stage")
        warm = const_pool.tile([P, 1], F32, tag="warm")
        warm2 = const_pool.tile([P, 1], F32, tag="warm2")

        # Post the input DMAs first (alternating between the two HWDGE
        # rings), interleaving const setup after the first posting so the
        # transfers start as early as possible.
        raws = []
        r0 = 0
        for si, ch in enumerate(STRIPS):
            raw = io_pool.tile([P, 4 * ch], F16, tag=f"raw{ch}", name=f"raw{si}")
            raw3 = raw[:].rearrange("p (q c) -> p q c", q=4)
            # Alternate the two HWDGE rings (SP and ACT sequencers) so two
            # strip transfers stream in parallel.
            eng = nc.sync if si % 2 == 0 else nc.scalar
            eng.dma_start(out=raw3, in_=xv[:, 4 * r0 : 4 * (r0 + ch)])
            raws.append((raw, r0, ch))
            r0 += ch
            if si == 0:
                make_upper_triangular(nc, U[:], val=1.0, diag=True)
                nc.gpsimd.memset(ONES[:], 1.0)
                # Warm the ACT Square table so the ~1.3us table load
                # overlaps the first input DMA.
                nc.vector.memset(warm[:], 0)
                nc.scalar.activation(
                    out=warm2[:],
                    in_=warm[:],
                    func=mybir.ActivationFunctionType.Square,
                )

        sup = 0
        for si, (raw, r0, ch) in enumerate(raws):
            z = z_pool.tile([P, 2 * ch], F16, tag=f"z{ch}", name=f"z{si}")
            # z1 = x1 - y1 (bins 0..127), z2 = x2 - y2 (bins 128..255)
            nc.vector.tensor_tensor(
                out=z[:, :ch],
                in0=raw[:, :ch],
                in1=raw[:, 2 * ch : 3 * ch],
                op=mybir.AluOpType.add,
            )
            nc.vector.tensor_tensor(
                out=z[:, ch:],
                in0=raw[:, ch : 2 * ch],
                in1=raw[:, 3 * ch :],
                op=mybir.AluOpType.add,
            )
            CH = ch  # strip-local width for the slices below
            for ui in range(ch // SUP):
                c0 = ui * SUP
                # Per-chunk C tiles (one PSUM bank each, triple-buffered) so
                # the next chunk's matmuls never wait on ACT draining the
                # previous C tile. U-stationary matmuls still batched first.
                C1s, C2s, z1s = [], [], []
                for k in range(2):
                    z1 = z[:, c0 + k * NCH : c0 + (k + 1) * NCH]
                    z2 = z[:, CH + c0 + k * NCH : CH + c0 + (k + 1) * NCH]
                    C1 = c1_pool.tile([P, NCH], F32, tag="C1")
                    C2 = c2_pool.tile([P, NCH], F32, tag="C2")
                    nc.tensor.matmul(C1[:], U[:], z1, start=True, stop=True)
                    nc.tensor.matmul(C2[:], U[:], z2, start=True, stop=False)
                    C1s.append(C1)
                    C2s.append(C2)
                    z1s.append(z1)
                for k in range(2):
                    nc.tensor.matmul(
                        C2s[k][:], ONES[:], z1s[k], start=False, stop=True
                    )
                # Reduce over bins: chunk 2u -> S partition 0, 2u+1 -> 64.
                S = s_pool.tile([P, NCH], F32, tag="S")
                for k in range(2):
                    sq1 = sq_pool.tile([P, NCH], F16, tag="sq1")
                    sq2 = sq_pool.tile([P, NCH], F16, tag="sq2")
                    nc.scalar.activation(
                        out=sq1[:],
                        in_=C1s[k][:],
                        func=mybir.ActivationFunctionType.Square,
                    )
                    nc.scalar.activation(
                        out=sq2[:],
                        in_=C2s[k][:],
                        func=mybir.ActivationFunctionType.Square,
                    )
                    # sq12 = sq1 + sq2 on DVE (2x fp16) halves the PE
                    # reduce matmuls.
                    sq12 = sq_pool.tile([P, NCH], F16, tag="sq12")
                    nc.vector.tensor_tensor(
                        out=sq12[:], in0=sq1[:], in1=sq2[:],
                        op=mybir.AluOpType.add,
                    )
                    off = 64 * k
                    nc.tensor.matmul(
                        S[off : off + 1, :], ONES[:, 0:1], sq12[:],
                        start=True, stop=True,
                    )
                nc.vector.tensor_copy(stage[:, sup, :], S[:])
                sup += 1
                if sup == N_SUP // 2:
                    # First half of the output can ship mid-kernel.
                    ov = out[:].rearrange("(n two c) -> two n c", two=2, c=NCH)
                    nc.sync.dma_start(
                        out=ov[0:1, : N_SUP // 2], in_=stage[0:1, : N_SUP // 2, :]
                    )
                    nc.sync.dma_start(
                        out=ov[1:2, : N_SUP // 2], in_=stage[64:65, : N_SUP // 2, :]
                    )
        # stage rows {0, 64} of staging slot u hold chunks 2u and 2u+1.
        ov = out[:].rearrange("(n two c) -> two n c", two=2, c=NCH)
        nc.sync.dma_start(out=ov[0:1, N_SUP // 2 :], in_=stage[0:1, N_SUP // 2 :, :])
        nc.sync.dma_start(out=ov[1:2, N_SUP // 2 :], in_=stage[64:65, N_SUP // 2 :, :])
    nc.finalize()
    return nc


_NC = None


def _get_nc() -> bass.Bass:
    global _NC
    if _NC is None:
        _NC = build_nc()
    return _NC


def make_in_maps(x: np.ndarray, y: np.ndarray) -> list[dict]:
    x16 = x.astype(np.float16)
    ny16 = (-y).astype(np.float16)
    in_maps = []
    for i in range(N_CORES):
        sl = slice(i * ROWS, (i + 1) * ROWS)
        # [2(t), 2(h), P, ROWS] -> flat strip-major [P, 4*ROWS]: per
        # partition, each strip contributes its 4 quadrant runs in order.
        xt = np.ascontiguousarray(x16[sl].T).reshape(2, P, ROWS)
        nyt = np.ascontiguousarray(ny16[sl].T).reshape(2, P, ROWS)
        q = np.stack([xt, nyt]).reshape(4, P, ROWS)
        flat = np.empty((P, 4 * ROWS), np.float16)
        r0 = 0
        for ch in STRIPS:
            flat[:, 4 * r0 : 4 * (r0 + ch)] = (
                q[:, :, r0 : r0 + ch].transpose(1, 0, 2).reshape(P, 4 * ch)
            )
            r0 += ch
        in_maps.append({"xy": flat})
    return in_maps


def kernel(x: np.ndarray, y: np.ndarray) -> np.ndarray:
    assert x.shape == (B, BINS) and y.shape == (B, BINS), (x.shape, y.shape)
    x = np.ascontiguousarray(x, dtype=np.float32)
    y = np.ascontiguousarray(y, dtype=np.float32)
    res = run_bass_kernel_spmd(_get_nc(), make_in_maps(x, y), list(range(N_CORES)))
    return np.concatenate([m["out"] for m in res.results])


# revision 33
# speedup vs baseline: 1.2664x; 1.2664x over previous
"""EMD loss kernel for Trainium2 (8 NeuronCores, pure data parallel).

Computes out[b] = sum_t (cumsum(x-y, axis=1)[b, t])^2 for x, y [131072, 256] f32.

Transposed matmul design (v3). The row-major scan design was a three-way tie
(DVE tensor_tensor_scan 667ns/block = 85us, ACT square+accum 739ns/block =
95us, f32 DMA 83us, exec 101us). This version:
  - uploads x and -y as fp16 in a bins-on-partitions transposed layout
    (16.8 MB/core, one packed HWDGE DMA per 2048-row strip).
  - DVE pre-adds z = x + (-y) in fp16 2x mode (z1 = bins 0..127 on the 128
    partitions, z2 = bins 128..255).
  - PE computes the cumulative-sum differences as triangular matmuls
    (C1 = U^T z1; C2 = U^T z2 + ONES^T z1; 3 matmuls per 512-row chunk),
    replacing the unscalable DVE scan.
  - ACT squares two chunks at a time PSUM->SBUF fp16 ([128, 1024] tiles to
    amortize the 172-cycle PSUM access bubble).
  - PE ones-vector matmuls reduce over bins into [1, 512] PSUM rows; two
    chunks share one S bank at partition offsets {0, 64}, halving the DVE
    PSUM->SBUF copies. A single final DMA scatters the staging tile to DRAM.
"""

import numpy as np

from concourse import bacc, bass, mybir
from concourse.bass_utils import run_bass_kernel_spmd
from concourse.masks import make_upper_triangular
from concourse.tile import TileContext

N_CORES = 8
B = 131072
BINS = 256
ROWS = B // N_CORES  # 16384 rows per core
P = 128
CH = 2048  # main strip width (rows per input DMA)
# Tapered strips: small head so compute starts early, small tails so the
# serial post-last-DMA compute is short. All multiples of SUP.
STRIPS = [2048] * 6 + [1024] * 4
assert sum(STRIPS) == ROWS
NCH = 512  # matmul moving free dim (chunk)
SUP = 1024  # super-chunk: ACT square granularity (2 chunks)
N_SUP = ROWS // SUP  # 16

F32 = mybir.dt.float32
F16 = mybir.dt.float16


def build_nc() -> bass.Bass:
    nc = bacc.Bacc()

    # Strip-major host layout: per (partition, strip) all four quadrants
    # (x/ny x binhalf) are contiguous, so each strip DMA is one long run
    # per partition instead of four short ones.
    xy = nc.declare_dram_parameter("xy", [P, 4 * ROWS], F16, isOutput=False)
    out = nc.declare_dram_parameter("out", [ROWS], F32, isOutput=True)
    xv = xy[:]

    with (
        TileContext(nc) as tc,
        tc.tile_pool(name="io", bufs=5) as io_pool,
        tc.tile_pool(name="zp", bufs=2) as z_pool,
        tc.tile_pool(name="sq", bufs=3) as sq_pool,
        tc.tile_pool(name="c1p", bufs=3, space="PSUM") as c1_pool,
        tc.tile_pool(name="c2p", bufs=3, space="PSUM") as c2_pool,
        tc.tile_pool(name="sp", bufs=2, space="PSUM") as s_pool,
        tc.tile_pool(name="const", bufs=1) as const_pool,
    ):
        U = const_pool.tile([P, P], F16, tag="U")
        ONES = const_pool.tile([P, P], F16, tag="ONES")
        stage = const_pool.tile([P, N_SUP, NCH], F32, tag="stage")
        warm = const_pool.tile([P, 1], F32, tag="warm")
        warm2 = const_pool.tile([P, 1], F32, tag="warm2")

        # Post the input DMAs first (alternating between the two HWDGE
        # rings), interleaving const setup after the first posting so the
        # transfers start as early as possible.
        raws = []
        r0 = 0
        for si, ch in enumerate(STRIPS):
            raw = io_pool.tile([P, 4 * ch], F16, tag=f"raw{ch}", name=f"raw{si}")
            raw3 = raw[:].rearrange("p (q c) -> p q c", q=4)
            nc.sync.dma_start(out=raw3, in_=xv[:, 4 * r0 : 4 * (r0 + ch)])
            raws.append((raw, r0, ch))
            r0 += ch
            if si == 0:
                make_upper_triangular(nc, U[:], val=1.0, diag=True)
                nc.gpsimd.memset(ONES[:], 1.0)
                # Warm the ACT Square table so the ~1.3us table load
                # overlaps the first input DMA.
                nc.vector.memset(warm[:], 0)
                nc.scalar.activation(
                    out=warm2[:],
                    in_=warm[:],
                    func=mybir.ActivationFunctionType.Square,
                )
                # ~3us of back-to-back dummy matmuls while the first input
                # DMA streams, ramping the PE clock out of its low p-state
                # before the real matmuls arrive.
                wpsum = s_pool.tile([P, NCH], F32, tag="S")
                for _ in range(16):
                    nc.tensor.matmul(
                        wpsum[:, :P], U[:], ONES[:], start=True, stop=True
                    )

        sup = 0
        for si, (raw, r0, ch) in enumerate(raws):
            z = z_pool.tile([P, 2 * ch], F16, tag=f"z{ch}", name=f"z{si}")
            # z1 = x1 - y1 (bins 0..127), z2 = x2 - y2 (bins 128..255)
            nc.vector.tensor_tensor(
                out=z[:, :ch],
                in0=raw[:, :ch],
                in1=raw[:, 2 * ch : 3 * ch],
                op=mybir.AluOpType.add,
            )
            nc.vector.tensor_tensor(
                out=z[:, ch:],
                in0=raw[:, ch : 2 * ch],
                in1=raw[:, 3 * ch :],
                op=mybir.AluOpType.add,
            )
            CH = ch  # strip-local width for the slices below
            for ui in range(ch // SUP):
                c0 = ui * SUP
                # Per-chunk C tiles (one PSUM bank each, triple-buffered) so
                # the next chunk's matmuls never wait on ACT draining the
                # previous C tile. U-stationary matmuls still batched first.
                C1s, C2s, z1s = [], [], []
                for k in range(2):
                    z1 = z[:, c0 + k * NCH : c0 + (k + 1) * NCH]
                    z2 = z[:, CH + c0 + k * NCH : CH + c0 + (k + 1) * NCH]
                    C1 = c1_pool.tile([P, NCH], F32, tag="C1")
                    C2 = c2_pool.tile([P, NCH], F32, tag="C2")
                    nc.tensor.matmul(C1[:], U[:], z1, start=True, stop=True)
                    nc.tensor.matmul(C2[:], U[:], z2, start=True, stop=False)
                    C1s.append(C1)
                    C2s.append(C2)
                    z1s.append(z1)
                for k in range(2):
                    nc.tensor.matmul(
                        C2s[k][:], ONES[:], z1s[k], start=False, stop=True
                    )
                # Reduce over bins: chunk 2u -> S partition 0, 2u+1 -> 64.
                S = s_pool.tile([P, NCH], F32, tag="S")
                for k in range(2):
                    sq1 = sq_pool.tile([P, NCH], F16, tag="sq1")
                    sq2 = sq_pool.tile([P, NCH], F16, tag="sq2")
                    nc.scalar.activation(
                        out=sq1[:],
                        in_=C1s[k][:],
                        func=mybir.ActivationFunctionType.Square,
                    )
                    nc.scalar.activation(
                        out=sq2[:],
                        in_=C2s[k][:],
                        func=mybir.ActivationFunctionType.Square,
                    )
                    # sq12 = sq1 + sq2 on DVE (2x fp16) halves the PE
                    # reduce matmuls.
                    sq12 = sq_pool.tile([P, NCH], F16, tag="sq12")
                    nc.vector.tensor_tensor(
                        out=sq12[:], in0=sq1[:], in1=sq2[:],
                        op=mybir.AluOpType.add,
                    )
                    off = 64 * k
                    nc.tensor.matmul(
                        S[off : off + 1, :], ONES[:, 0:1], sq12[:],
                        start=True, stop=True,
                    )
                nc.vector.tensor_copy(stage[:, sup, :], S[:])
                sup += 1
                if sup == N_SUP // 2:
                    # First half of the output can ship mid-kernel.
                    ov = out[:].rearrange("(n two c) -> two n c", two=2, c=NCH)
                    nc.sync.dma_start(
                        out=ov[0:1, : N_SUP // 2], in_=stage[0:1, : N_SUP // 2, :]
                    )
                    nc.sync.dma_start(
                        out=ov[1:2, : N_SUP // 2], in_=stage[64:65, : N_SUP // 2, :]
                    )
        # stage rows {0, 64} of staging slot u hold chunks 2u and 2u+1.
        ov = out[:].rearrange("(n two c) -> two n c", two=2, c=NCH)
        nc.sync.dma_start(out=ov[0:1, N_SUP // 2 :], in_=stage[0:1, N_SUP // 2 :, :])
        nc.sync.dma_start(out=ov[1:2, N_SUP // 2 :], in_=stage[64:65, N_SUP // 2 :, :])
    nc.finalize()
    return nc


_NC = None


def _get_nc() -> bass.Bass:
    global _NC
    if _NC is None:
        _NC = build_nc()
    return _NC


def make_in_maps(x: np.ndarray, y: np.ndarray) -> list[dict]:
    x16 = x.astype(np.float16)
    ny16 = (-y).astype(np.float16)
    in_maps = []
    for i in range(N_CORES):
        sl = slice(i * ROWS, (i + 1) * ROWS)
        # [2(t), 2(h), P, ROWS] -> flat strip-major [P, 4*ROWS]: per
        # partition, each strip contributes its 4 quadrant runs in order.
        xt = np.ascontiguousarray(x16[sl].T).reshape(2, P, ROWS)
        nyt = np.ascontiguousarray(ny16[sl].T).reshape(2, P, ROWS)
        q = np.stack([xt, nyt]).reshape(4, P, ROWS)
        flat = np.empty((P, 4 * ROWS), np.float16)
        r0 = 0
        for ch in STRIPS:
            flat[:, 4 * r0 : 4 * (r0 + ch)] = (
                q[:, :, r0 : r0 + ch].transpose(1, 0, 2).reshape(P, 4 * ch)
            )
            r0 += ch
        in_maps.append({"xy": flat})
    return in_maps


def kernel(x: np.ndarray, y: np.ndarray) -> np.ndarray:
    assert x.shape == (B, BINS) and y.shape == (B, BINS), (x.shape, y.shape)
    x = np.ascontiguousarray(x, dtype=np.float32)
    y = np.ascontiguousarray(y, dtype=np.float32)
    res = run_bass_kernel_spmd(_get_nc(), make_in_maps(x, y), list(range(N_CORES)))
    return np.concatenate([m["out"] for m in res.results])


# revision 34
# speedup vs baseline: 1.3000x; 1.0265x over previous
"""EMD loss kernel for Trainium2 (8 NeuronCores, pure data parallel).

Computes out[b] = sum_t (cumsum(x-y, axis=1)[b, t])^2 for x, y [131072, 256] f32.

Transposed matmul design (v3). The row-major scan design was a three-way tie
(DVE tensor_tensor_scan 667ns/block = 85us, ACT square+accum 739ns/block =
95us, f32 DMA 83us, exec 101us). This version:
  - uploads x and -y as fp16 in a bins-on-partitions transposed layout
    (16.8 MB/core, one packed HWDGE DMA per 2048-row strip).
  - DVE pre-adds z = x + (-y) in fp16 2x mode (z1 = bins 0..127 on the 128
    partitions, z2 = bins 128..255).
  - PE computes the cumulative-sum differences as triangular matmuls
    (C1 = U^T z1; C2 = U^T z2 + ONES^T z1; 3 matmuls per 512-row chunk),
    replacing the unscalable DVE scan.
  - ACT squares two chunks at a time PSUM->SBUF fp16 ([128, 1024] tiles to
    amortize the 172-cycle PSUM access bubble).
  - PE ones-vector matmuls reduce over bins into [1, 512] PSUM rows; two
    chunks share one S bank at partition offsets {0, 64}, halving the DVE
    PSUM->SBUF copies. A single final DMA scatters the staging tile to DRAM.
"""

import numpy as np

from concourse import bacc, bass, mybir
from concourse.bass_utils import run_bass_kernel_spmd
from concourse.masks import make_upper_triangular
from concourse.tile import TileContext

N_CORES = 8
B = 131072
BINS = 256
ROWS = B // N_CORES  # 16384 rows per core
P = 128
CH = 2048  # main strip width (rows per input DMA)
# Tapered strips: small head so compute starts early, small tails so the
# serial post-last-DMA compute is short. All multiples of SUP.
STRIPS = [2048] * 6 + [1024] * 4
assert sum(STRIPS) == ROWS
NCH = 512  # matmul moving free dim (chunk)
SUP = 1024  # super-chunk: ACT square granularity (2 chunks)
N_SUP = ROWS // SUP  # 16

F32 = mybir.dt.float32
F16 = mybir.dt.float16


def build_nc() -> bass.Bass:
    nc = bacc.Bacc()

    # Strip-major host layout: per (partition, strip) all four quadrants
    # (x/ny x binhalf) are contiguous, so each strip DMA is one long run
    # per partition instead of four short ones.
    xy = nc.declare_dram_parameter("xy", [P, 4 * ROWS], F16, isOutput=False)
    out = nc.declare_dram_parameter("out", [ROWS], F32, isOutput=True)
    xv = xy[:]

    with (
        TileContext(nc) as tc,
        tc.tile_pool(name="io", bufs=5) as io_pool,
        tc.tile_pool(name="zp", bufs=3) as z_pool,
        tc.tile_pool(name="sq", bufs=3) as sq_pool,
        tc.tile_pool(name="c1p", bufs=3, space="PSUM") as c1_pool,
        tc.tile_pool(name="c2p", bufs=3, space="PSUM") as c2_pool,
        tc.tile_pool(name="sp", bufs=2, space="PSUM") as s_pool,
        tc.tile_pool(name="const", bufs=1) as const_pool,
    ):
        U = const_pool.tile([P, P], F16, tag="U")
        ONES = const_pool.tile([P, P], F16, tag="ONES")
        stage = const_pool.tile([P, N_SUP, NCH], F32, tag="stage")
        warm = const_pool.tile([P, 1], F32, tag="warm")
        warm2 = const_pool.tile([P, 1], F32, tag="warm2")

        # Post the input DMAs first (alternating between the two HWDGE
        # rings), interleaving const setup after the first posting so the
        # transfers start as early as possible.
        raws = []
        r0 = 0
        for si, ch in enumerate(STRIPS):
            raw = io_pool.tile([P, 4 * ch], F16, tag=f"raw{ch}", name=f"raw{si}")
            raw3 = raw[:].rearrange("p (q c) -> p q c", q=4)
            nc.sync.dma_start(out=raw3, in_=xv[:, 4 * r0 : 4 * (r0 + ch)])
            raws.append((raw, r0, ch))
            r0 += ch
            if si == 0:
                make_upper_triangular(nc, U[:], val=1.0, diag=True)
                nc.gpsimd.memset(ONES[:], 1.0)
                # Warm the ACT Square table so the ~1.3us table load
                # overlaps the first input DMA.
                nc.vector.memset(warm[:], 0)
                nc.scalar.activation(
                    out=warm2[:],
                    in_=warm[:],
                    func=mybir.ActivationFunctionType.Square,
                )
                # ~3us of back-to-back dummy matmuls while the first input
                # DMA streams, ramping the PE clock out of its low p-state
                # before the real matmuls arrive.
                wpsum = s_pool.tile([P, NCH], F32, tag="S")
                for _ in range(16):
                    nc.tensor.matmul(
                        wpsum[:, :P], U[:], ONES[:], start=True, stop=True
                    )

        sup = 0
        for si, (raw, r0, ch) in enumerate(raws):
            z = z_pool.tile([P, 2 * ch], F16, tag=f"z{ch}", name=f"z{si}")
            # z1 = x1 - y1 (bins 0..127), z2 = x2 - y2 (bins 128..255)
            nc.vector.tensor_tensor(
                out=z[:, :ch],
                in0=raw[:, :ch],
                in1=raw[:, 2 * ch : 3 * ch],
                op=mybir.AluOpType.add,
            )
            nc.vector.tensor_tensor(
                out=z[:, ch:],
                in0=raw[:, ch : 2 * ch],
                in1=raw[:, 3 * ch :],
                op=mybir.AluOpType.add,
            )
            CH = ch  # strip-local width for the slices below
            for ui in range(ch // SUP):
                c0 = ui * SUP
                # Per-chunk C tiles (one PSUM bank each, triple-buffered) so
                # the next chunk's matmuls never wait on ACT draining the
                # previous C tile. U-stationary matmuls still batched first.
                C1s, C2s, z1s = [], [], []
                for k in range(2):
                    z1 = z[:, c0 + k * NCH : c0 + (k + 1) * NCH]
                    z2 = z[:, CH + c0 + k * NCH : CH + c0 + (k + 1) * NCH]
                    C1 = c1_pool.tile([P, NCH], F32, tag="C1")
                    C2 = c2_pool.tile([P, NCH], F32, tag="C2")
                    nc.tensor.matmul(C1[:], U[:], z1, start=True, stop=True)
                    nc.tensor.matmul(C2[:], U[:], z2, start=True, stop=False)
                    C1s.append(C1)
                    C2s.append(C2)
                    z1s.append(z1)
                for k in range(2):
                    nc.tensor.matmul(
                        C2s[k][:], ONES[:], z1s[k], start=False, stop=True
                    )
                # Reduce over bins: chunk 2u -> S partition 0, 2u+1 -> 64.
                S = s_pool.tile([P, NCH], F32, tag="S")
                for k in range(2):
                    sq1 = sq_pool.tile([P, NCH], F16, tag="sq1")
                    sq2 = sq_pool.tile([P, NCH], F16, tag="sq2")
                    nc.scalar.activation(
                        out=sq1[:],
                        in_=C1s[k][:],
                        func=mybir.ActivationFunctionType.Square,
                    )
                    nc.scalar.activation(
                        out=sq2[:],
                        in_=C2s[k][:],
                        func=mybir.ActivationFunctionType.Square,
                    )
                    # sq12 = sq1 + sq2 on DVE (2x fp16) halves the PE
                    # reduce matmuls.
                    sq12 = sq_pool.tile([P, NCH], F16, tag="sq12")
                    nc.vector.tensor_tensor(
                        out=sq12[:], in0=sq1[:], in1=sq2[:],
                        op=mybir.AluOpType.add,
                    )
                    off = 64 * k
                    nc.tensor.matmul(
                        S[off : off + 1, :], ONES[:, 0:1], sq12[:],
                        start=True, stop=True,
                    )
                nc.vector.tensor_copy(stage[:, sup, :], S[:])
                sup += 1
                if sup == N_SUP // 2:
                    # First half of the output can ship mid-kernel.
                    ov = out[:].rearrange("(n two c) -> two n c", two=2, c=NCH)
                    nc.sync.dma_start(
                        out=ov[0:1, : N_SUP // 2], in_=stage[0:1, : N_SUP // 2, :]
                    )
                    nc.sync.dma_start(
                        out=ov[1:2, : N_SUP // 2], in_=stage[64:65, : N_SUP // 2, :]
                    )
        # stage rows {0, 64} of staging slot u hold chunks 2u and 2u+1.
        ov = out[:].rearrange("(n two c) -> two n c", two=2, c=NCH)
        nc.sync.dma_start(out=ov[0:1, N_SUP // 2 :], in_=stage[0:1, N_SUP // 2 :, :])
        nc.sync.dma_start(out=ov[1:2, N_SUP // 2 :], in_=stage[64:65, N_SUP // 2 :, :])
    nc.finalize()
    return nc


_NC = None


def _get_nc() -> bass.Bass:
    global _NC
    if _NC is None:
        _NC = build_nc()
    return _NC


def make_in_maps(x: np.ndarray, y: np.ndarray) -> list[dict]:
    x16 = x.astype(np.float16)
    ny16 = (-y).astype(np.float16)
    in_maps = []
    for i in range(N_CORES):
        sl = slice(i * ROWS, (i + 1) * ROWS)
        # [2(t), 2(h), P, ROWS] -> flat strip-major [P, 4*ROWS]: per
        # partition, each strip contributes its 4 quadrant runs in order.
        xt = np.ascontiguousarray(x16[sl].T).reshape(2, P, ROWS)
        nyt = np.ascontiguousarray(ny16[sl].T).reshape(2, P, ROWS)
        q = np.stack([xt, nyt]).reshape(4, P, ROWS)
        flat = np.empty((P, 4 * ROWS), np.float16)
        r0 = 0
        for ch in STRIPS:
            flat[:, 4 * r0 : 4 * (r0 + ch)] = (
                q[:, :, r0 : r0 + ch].transpose(1, 0, 2).reshape(P, 4 * ch)
            )
            r0 += ch
        in_maps.append({"xy": flat})
    return in_maps


def kernel(x: np.ndarray, y: np.ndarray) -> np.ndarray:
    assert x.shape == (B, BINS) and y.shape == (B, BINS), (x.shape, y.shape)
    x = np.ascontiguousarray(x, dtype=np.float32)
    y = np.ascontiguousarray(y, dtype=np.float32)
    res = run_bass_kernel_spmd(_get_nc(), make_in_maps(x, y), list(range(N_CORES)))
    return np.concatenate([m["out"] for m in res.results])
